# revision 18
# baseline (speedup 1.0000x reference)
"""Trainium2 Bass kernel for nn_MAR_52209622450490 (OctFormer sparse attention).

Sharding: the depth2batch gather is applied host-side while sharding — each
core gets a contiguous 2048-token slice of the *window-ordered* token stream.
2048 is a multiple of the 512-token super-window (P*DIL), so both the dense
and the dilated window partitions are fully core-local; no collectives needed.
The 4 transformer blocks plus both loss heads run on-device per core; each
core emits 4 partial sums (ce_s*ms, ms, ce_v*mv, mv) combined on host.

Device layout notes:
 - residual stream XB kept [token(part) x C(free)] f32; LN via bn_stats.
 - LN scale/bias folded into the following matmul's weights host-side.
 - matmul operands in bf16; PE transposes (via identity) produce the
   channel-on-partition operands (HT/OT) needed as lhsT/rhs.
 - attention: scores-transposed S'[kq, p] via 4-head row-tiled K=32 matmuls
   (tile_position); exp on ACT (no max-subtraction needed: logits are O(0.1));
   AV matmul uses V with an appended ones-column so each output tile carries
   its softmax normalizer Z; normalization fused into PSUM evacuation.
 - dilated windows are pure strided APs on the token (free) axis.
"""
import numpy as np
import ml_dtypes

import concourse.tile as tile
from concourse import bacc, mybir
from concourse.bass_utils import run_bass_kernel_spmd
from concourse.masks import make_identity

N_SPLIT = 4096
N_VQ = 12288
N = N_SPLIT + N_VQ
C = 256
H = 8
DH = 32
L = 4
P = 256
DIL = 2
HID = 4 * C
VQ_G = 4
VQ_SIZE = 256
NCORES = 8
T = N // NCORES            # 2048 tokens per core
TC = T // 128              # 16 row-tiles per core
NWIN = T // P              # 8 windows per core
EPS = 1e-5
SCALE = DH ** -0.5

F32 = mybir.dt.float32
BF16 = mybir.dt.bfloat16
BF = ml_dtypes.bfloat16

_CACHE = {}


def _sin_pos_emb(n, c):
    pos = np.arange(n, dtype=np.float32)[:, None]
    half = c // 2
    freqs = np.exp(-np.log(10000.0) * np.arange(half, dtype=np.float32) / half)
    ang = pos * freqs
    return np.concatenate([np.sin(ang), np.cos(ang)], axis=-1).astype(np.float32)



def _st(beg, cnt, step):
    return slice(beg, beg + (cnt - 1) * step + 1, step)

def build_nc(flags, n_blocks=L, dump=None):
    """flags: dict name->bool, whether each bias family is nonzero.
    dump: None|'xb'|'xn' adds a [T, C] f32 debug output."""
    nc = bacc.Bacc(None, target_bir_lowering=False)

    d_emb = nc.declare_dram_parameter("emb", [T, C], F32, isOutput=False)
    d_zqt = nc.declare_dram_parameter("zqt", [DH, T], BF16, isOutput=False)
    d_vqpw = nc.declare_dram_parameter("vqpw", [DH, C], BF16, isOutput=False)
    d_wqkv = nc.declare_dram_parameter("wqkv", [L, C, 3 * C], BF16, isOutput=False)
    d_wattn = nc.declare_dram_parameter("wattn", [L, C, C], BF16, isOutput=False)
    d_wfc1 = nc.declare_dram_parameter("wfc1", [L, C, HID], BF16, isOutput=False)
    d_wfc2 = nc.declare_dram_parameter("wfc2", [L, HID, C], BF16, isOutput=False)
    d_bqkv = nc.declare_dram_parameter("bqkv", [L, 3 * C], F32, isOutput=False)
    d_battn = nc.declare_dram_parameter("battn", [L, C], F32, isOutput=False)
    d_bfc1 = nc.declare_dram_parameter("bfc1", [L, HID], F32, isOutput=False)
    d_bfc2 = nc.declare_dram_parameter("bfc2", [L, C], F32, isOutput=False)
    d_wvq = nc.declare_dram_parameter("wvq", [C, VQ_G * VQ_SIZE], BF16, isOutput=False)
    d_wspl = nc.declare_dram_parameter("wspl", [C, 2], BF16, isOutput=False)
    d_bspl = nc.declare_dram_parameter("bspl", [2], F32, isOutput=False)
    d_ebq = nc.declare_dram_parameter("ebq", [VQ_G * VQ_SIZE], F32, isOutput=False)
    d_wsel = nc.declare_dram_parameter("wsel", [T, C], BF16, isOutput=False)
    d_bsel = nc.declare_dram_parameter("bsel", [T], F32, isOutput=False)
    d_msc = nc.declare_dram_parameter("msc", [T], F32, isOutput=False)
    d_mvc = nc.declare_dram_parameter("mvc", [T], F32, isOutput=False)
    d_stc = nc.declare_dram_parameter("stc", [T], F32, isOutput=False)
    d_out = nc.declare_dram_parameter("out", [128, 4], F32, isOutput=True)
    d_dbg = None
    if dump is not None:
        d_dbg = nc.declare_dram_parameter("dbg", [T, C], F32, isOutput=True)

    with tile.TileContext(nc) as tc:
        with (
            tc.tile_pool(name="big", bufs=1) as big,
            tc.tile_pool(name="wpool", bufs=2) as wp,
            tc.tile_pool(name="small", bufs=1) as sm,
            tc.tile_pool(name="trans", bufs=6) as tr,
            tc.tile_pool(name="ebpool", bufs=4) as ebp,
            tc.tile_pool(name="evpool", bufs=3) as evp,
            tc.tile_pool(name="psum", bufs=3, space="PSUM") as psp,
            tc.tile_pool(name="psum_sc", bufs=2, space="PSUM") as psc,
            tc.tile_pool(name="psum_av", bufs=1, space="PSUM") as pav,
            tc.tile_pool(name="psum_tr", bufs=2, space="PSUM") as ptr,
        ):
            XB = big.tile([128, TC, C], F32, tag="XB")
            HT = big.tile([128, 2, T], BF16, tag="HT")
            QT = big.tile([128, 2, T], BF16, tag="QT")
            KT = big.tile([128, 2, T], BF16, tag="KT")
            VB = big.tile([128, TC, H, DH + 1], BF16, tag="VB")
            OROW = big.tile([128, TC, C], BF16, tag="OROW")
            OT = big.tile([128, 2, T], BF16, tag="OT")
            GT = big.tile([128, HID // 128, T], BF16, tag="GT")
            XN = big.tile([128, TC, C], BF16, tag="XN")

            ident = sm.tile([128, 128], BF16, tag="ident")
            make_identity(nc, ident[:])
            epsT = sm.tile([128, 1], F32, tag="eps")
            nc.vector.memset(epsT[:], EPS)
            zqt = sm.tile([DH, T], BF16, tag="zqt")
            nc.sync.dma_start(zqt[:], d_zqt[:])
            vqpw = sm.tile([DH, C], BF16, tag="vqpw")
            nc.sync.dma_start(vqpw[:], d_vqpw[:])

            nc.vector.memset(VB[:, :, :, DH], 1.0)

            # ---------------- embed ----------------
            nc.sync.dma_start(XB[:], d_emb.rearrange("(t p) c -> p t c", p=128))
            for t in range(TC):
                ps = psp.tile([128, 512], F32, tag="bank")
                nc.tensor.matmul(ps[:, :C], zqt[:, t * 128:(t + 1) * 128],
                                 vqpw[:], start=True, stop=True)
                nc.vector.tensor_add(XB[:, t, :], XB[:, t, :], ps[:, :C])

            def layernorm_to(dst_bf, t):
                st6 = tr.tile([128, 6], F32, tag="bn6")
                nc.vector.bn_stats(st6[:], XB[:, t, :])
                mv2 = tr.tile([128, 2], F32, tag="bn2")
                nc.vector.bn_aggr(mv2[:], st6[:])
                rstd = tr.tile([128, 1], F32, tag="rstd")
                nc.scalar.activation(rstd[:], mv2[:, 1:2],
                                     mybir.ActivationFunctionType.Sqrt,
                                     bias=epsT[:])
                nc.vector.reciprocal(rstd[:], rstd[:])
                nc.vector.tensor_scalar(dst_bf, XB[:, t, :],
                                        mv2[:, 0:1], rstd[:],
                                        mybir.AluOpType.subtract,
                                        mybir.AluOpType.mult)

            def dma_transpose_pair(dstT, t128, src_128x256):
                """contiguous-dest transpose via XBAR DMA (both c-chunks)."""
                for cc in range(2):
                    nc.sync.dma_start_transpose(
                        dstT[:, cc, t128:t128 + 128],
                        src_128x256[:, cc * 128:(cc + 1) * 128])

            def transpose_pair(dstT, tok_ap_2x128, src_128x256):
                """dstT[:, cc, tok] = src[:, cc*128:+128].T for cc in 0,1; one evac."""
                pst = ptr.tile([128, 512], BF16, tag="ps_tr")
                nc.tensor.transpose(pst[:, 0:128], src_128x256[:, 0:128], ident[:])
                nc.tensor.transpose(pst[:, 128:256], src_128x256[:, 128:256], ident[:])
                nc.vector.tensor_copy(tok_ap_2x128,
                                      pst[:, :256].rearrange("p (k n) -> p k n", k=2))

            # ---------------- transformer blocks ----------------
            for l in range(n_blocks):
                wqkv = wp.tile([128, 2, 3 * C], BF16, tag="wqkv")
                nc.sync.dma_start(wqkv[:], d_wqkv[l].rearrange("(k p) n -> p k n", p=128))
                wattn = wp.tile([128, 2, C], BF16, tag="wattn")
                nc.sync.dma_start(wattn[:], d_wattn[l].rearrange("(k p) n -> p k n", p=128))
                wfc1 = wp.tile([128, 2, HID], BF16, tag="wfc1")
                nc.sync.dma_start(wfc1[:], d_wfc1[l].rearrange("(k p) n -> p k n", p=128))
                wfc2 = wp.tile([128, HID // 128, C], BF16, tag="wfc2")
                nc.sync.dma_start(wfc2[:], d_wfc2[l].rearrange("(k p) n -> p k n", p=128))
                bqkv = vbb = abb = f1b = f2b = None
                if flags["bqkv"]:
                    bqkv = wp.tile([128, 4], F32, tag="bqkv")
                    nc.sync.dma_start(bqkv[:], d_bqkv[l, :2 * C].rearrange("(g p) -> p g", p=128))
                if flags["bqkv_v"]:
                    vbb = wp.tile([128, C], F32, tag="vbb")
                    nc.sync.dma_start(vbb[:], d_bqkv[l, 2 * C:].to_broadcast([128, C]))
                if flags["battn"]:
                    abb = wp.tile([128, C], F32, tag="abb")
                    nc.sync.dma_start(abb[:], d_battn[l].to_broadcast([128, C]))
                if flags["bfc1"]:
                    f1b = wp.tile([128, HID // 128], F32, tag="f1b")
                    nc.sync.dma_start(f1b[:], d_bfc1[l].rearrange("(g p) -> p g", p=128))
                if flags["bfc2"]:
                    f2b = wp.tile([128, C], F32, tag="f2b")
                    nc.sync.dma_start(f2b[:], d_bfc2[l].to_broadcast([128, C]))

                dil = DIL if (l % 2 == 1) else 1

                # LN1 -> hbf -> HT (h transposed)
                for t in range(TC):
                    hbf = tr.tile([128, C], BF16, tag="hbf")
                    layernorm_to(hbf[:], t)
                    dma_transpose_pair(HT, t * 128, hbf[:])

                # QT / KT (transposed q,k; head-major on partitions)
                for g in range(4):          # 0,1 -> Q ; 2,3 -> K
                    dstT = QT if g < 2 else KT
                    gg = g % 2
                    for nk in range(T // 512):
                        ps = psp.tile([128, 512], F32, tag="bank")
                        for cc in range(2):
                            nc.tensor.matmul(ps[:],
                                             wqkv[:, cc, g * 128:(g + 1) * 128],
                                             HT[:, cc, nk * 512:(nk + 1) * 512],
                                             start=(cc == 0), stop=(cc == 1))
                        dsl = dstT[:, gg, nk * 512:(nk + 1) * 512]
                        if flags["bqkv"]:
                            nc.scalar.activation(dsl, ps[:],
                                                 mybir.ActivationFunctionType.Identity,
                                                 bias=bqkv[:, g:g + 1])
                        elif nk % 2 == 0:
                            nc.vector.tensor_copy(dsl, ps[:])
                        else:
                            nc.scalar.copy(dsl, ps[:])

                # V in window order: VB[:, wlin*2+qc, h, 0:DH]
                for wlin in range(NWIN):
                    sw, r = divmod(wlin, dil)
                    start = sw * P * dil + r
                    for qc in range(2):
                        ps = psp.tile([128, 512], F32, tag="bank")
                        tok = _st(start + qc * 128 * dil, 128, dil)
                        for cc in range(2):
                            nc.tensor.matmul(ps[:, :C], HT[:, cc, tok],
                                             wqkv[:, cc, 2 * C:3 * C],
                                             start=(cc == 0), stop=(cc == 1))
                        vdst = VB[:, wlin * 2 + qc, :, 0:DH]
                        psv = ps[:, :C].rearrange("p (h d) -> p h d", h=H)
                        if flags["bqkv_v"]:
                            nc.vector.tensor_tensor(
                                vdst, psv,
                                vbb[:].rearrange("p (h d) -> p h d", h=H),
                                mybir.AluOpType.add)
                        else:
                            nc.vector.tensor_copy(vdst, psv)

                # attention per window
                for wlin in range(NWIN):
                    sw, r = divmod(wlin, dil)
                    start = sw * P * dil + r
                    alltok = _st(start, P, dil)
                    EB = ebp.tile([128, 2, H, P], BF16, tag="EB")
                    for g in range(2):
                        for hs in range(4):
                            hh = g * 4 + hs
                            prt = slice(hs * 32, (hs + 1) * 32)
                            pss = psc.tile([128, 512], F32, tag="ps_sc")
                            for qc in range(2):
                                ktok = _st(start + qc * 128 * dil, 128, dil)
                                nc.tensor.matmul(pss[:, qc * P:(qc + 1) * P],
                                                 KT[prt, g, ktok],
                                                 QT[prt, g, alltok],
                                                 start=True, stop=True,
                                                 tile_position=(hs * 32, 0))
                            nc.scalar.activation(
                                EB[:, :, hh, :],
                                pss[:].rearrange("p (q x) -> p q x", q=2),
                                mybir.ActivationFunctionType.Exp,
                                scale=SCALE)
                    for wc in range(2):
                        pso = pav.tile([128, H, DH + 1], F32, tag="ps_av")
                        for hh in range(H):
                            for qc in range(2):
                                nc.tensor.matmul(
                                    pso[:, hh, :],
                                    EB[:, qc, hh, wc * 128:(wc + 1) * 128],
                                    VB[:, wlin * 2 + qc, hh, :],
                                    start=(qc == 0), stop=(qc == 1))
                        rz = tr.tile([128, H], F32, tag="rz")
                        nc.vector.reciprocal(rz[:], pso[:, :, DH])
                        nc.vector.tensor_tensor(
                            OROW[:, wlin * 2 + wc, :].rearrange("p (h d) -> p h d", h=H),
                            pso[:, :, 0:DH],
                            rz[:, :, None].to_broadcast([128, H, DH]),
                            mybir.AluOpType.mult)
                        dtok = _st(start + wc * 128 * dil, 128, dil)
                        if dil == 1:
                            dma_transpose_pair(OT, start + wc * 128,
                                               OROW[:, wlin * 2 + wc, :])
                        else:
                            transpose_pair(OT, OT[:, :, dtok],
                                           OROW[:, wlin * 2 + wc, :])

                # proj + residual + LN2 + h2T, fused per t-chunk
                for t in range(TC):
                    ps = psp.tile([128, 512], F32, tag="bank")
                    for cc in range(2):
                        nc.tensor.matmul(ps[:, :C], OT[:, cc, t * 128:(t + 1) * 128],
                                         wattn[:, cc, :],
                                         start=(cc == 0), stop=(cc == 1))
                    nc.vector.tensor_add(XB[:, t, :], XB[:, t, :], ps[:, :C])
                    if flags["battn"]:
                        nc.vector.tensor_add(XB[:, t, :], XB[:, t, :], abb[:])
                    hbf = tr.tile([128, C], BF16, tag="hbf")
                    layernorm_to(hbf[:], t)
                    dma_transpose_pair(HT, t * 128, hbf[:])

                # fc1 + gelu -> GT (transposed, hid on partitions)
                for nk in range(T // 512):
                    for m in range(HID // 128):
                        ps = psp.tile([128, 512], F32, tag="bank")
                        for cc in range(2):
                            nc.tensor.matmul(ps[:],
                                             wfc1[:, cc, m * 128:(m + 1) * 128],
                                             HT[:, cc, nk * 512:(nk + 1) * 512],
                                             start=(cc == 0), stop=(cc == 1))
                        nc.scalar.activation(
                            GT[:, m, nk * 512:(nk + 1) * 512], ps[:],
                            mybir.ActivationFunctionType.Gelu_apprx_tanh,
                            bias=(f1b[:, m:m + 1] if flags["bfc1"] else 0.0))

                # fc2 + residual
                for t in range(TC):
                    ps = psp.tile([128, 512], F32, tag="bank")
                    for hc in range(HID // 128):
                        nc.tensor.matmul(ps[:, :C], GT[:, hc, t * 128:(t + 1) * 128],
                                         wfc2[:, hc, :],
                                         start=(hc == 0), stop=(hc == HID // 128 - 1))
                    nc.vector.tensor_add(XB[:, t, :], XB[:, t, :], ps[:, :C])
                    if flags["bfc2"]:
                        nc.vector.tensor_add(XB[:, t, :], XB[:, t, :], f2b[:])

            if dump == "xb":
                nc.sync.dma_start(d_dbg.rearrange("(t p) c -> p t c", p=128), XB[:])

            # ---------------- heads ----------------
            wvq = sm.tile([128, 2, VQ_G * VQ_SIZE], BF16, tag="wvq")
            nc.sync.dma_start(wvq[:], d_wvq.rearrange("(k p) n -> p k n", p=128))
            wspl = sm.tile([128, 2, 2], BF16, tag="wspl")
            nc.sync.dma_start(wspl[:], d_wspl.rearrange("(k p) n -> p k n", p=128))
            wselB = sm.tile([128, TC, C], BF16, tag="wsel")
            nc.sync.dma_start(wselB[:], d_wsel.rearrange("(t p) c -> p t c", p=128))
            MSC = sm.tile([128, TC], F32, tag="msc")
            nc.sync.dma_start(MSC[:], d_msc.rearrange("(t p) -> p t", p=128))
            MVC = sm.tile([128, TC], F32, tag="mvc")
            nc.sync.dma_start(MVC[:], d_mvc.rearrange("(t p) -> p t", p=128))
            STC = sm.tile([128, TC], F32, tag="stc")
            nc.sync.dma_start(STC[:], d_stc.rearrange("(t p) -> p t", p=128))
            if flags["bsel"]:
                BSL = sm.tile([128, TC], F32, tag="bsel")
                nc.sync.dma_start(BSL[:], d_bsel.rearrange("(t p) -> p t", p=128))
            if flags["ebq"]:
                EBQ = sm.tile([128, VQ_G * VQ_SIZE], F32, tag="ebq")
                nc.sync.dma_start(EBQ[:], d_ebq.to_broadcast([128, VQ_G * VQ_SIZE]))
            if flags["bspl"]:
                BSP = sm.tile([128, 2], F32, tag="bspl")
                nc.sync.dma_start(BSP[:], d_bspl.to_broadcast([128, 2]))

            SLB = sm.tile([128, TC, 2], F32, tag="SLB")
            GSL = sm.tile([128, TC, VQ_G], F32, tag="GSL")
            TSB = sm.tile([128, TC], F32, tag="TSB")

            # final LN -> XN (bf16) and XNT (reuse HT)
            for t in range(TC):
                layernorm_to(XN[:, t, :], t)
                dma_transpose_pair(HT, t * 128, XN[:, t, :])
            if dump == "xn":
                nc.gpsimd.dma_start(d_dbg.rearrange("(t p) c -> p t c", p=128), XN[:])

            for t in range(TC):
                EV = evp.tile([128, VQ_G * VQ_SIZE], F32, tag="EV")
                for nk in range(2):
                    ps = psp.tile([128, 512], F32, tag="bank")
                    for cc in range(2):
                        nc.tensor.matmul(ps[:],
                                         HT[:, cc, t * 128:(t + 1) * 128],
                                         wvq[:, cc, nk * 512:(nk + 1) * 512],
                                         start=(cc == 0), stop=(cc == 1))
                    nc.scalar.activation(EV[:, nk * 512:(nk + 1) * 512], ps[:],
                                         mybir.ActivationFunctionType.Exp)
                if flags["ebq"]:
                    nc.vector.tensor_tensor(EV[:], EV[:], EBQ[:],
                                            mybir.AluOpType.mult)
                g4 = tr.tile([128, VQ_G], F32, tag="g4")
                nc.vector.tensor_reduce(
                    g4[:],
                    EV[:].rearrange("p (g v) -> p g v", g=VQ_G),
                    mybir.AxisListType.X, mybir.AluOpType.add)
                nc.vector.tensor_copy(GSL[:, t, :], g4[:])

                ps2 = psp.tile([128, 512], F32, tag="bank")
                for cc in range(2):
                    nc.tensor.matmul(ps2[:, :2],
                                     HT[:, cc, t * 128:(t + 1) * 128],
                                     wspl[:, cc, :],
                                     start=(cc == 0), stop=(cc == 1))
                if flags["bspl"]:
                    nc.vector.tensor_tensor(SLB[:, t, :], ps2[:, :2], BSP[:],
                                            mybir.AluOpType.add)
                else:
                    nc.vector.tensor_copy(SLB[:, t, :], ps2[:, :2])

                tmp = tr.tile([128, C], F32, tag="wdot")
                nc.vector.tensor_tensor(tmp[:], XN[:, t, :], wselB[:, t, :],
                                        mybir.AluOpType.mult)
                rt1 = tr.tile([128, 1], F32, tag="rt1")
                nc.vector.tensor_reduce(rt1[:], tmp[:],
                                        mybir.AxisListType.X, mybir.AluOpType.add)
                nc.vector.tensor_copy(TSB[:, t:t + 1], rt1[:])

            # finish:  ce_v = 0.25*(sum_g ln GSL_g) - 0.25*(TSB [+bsel])
            LGS = sm.tile([128, TC, VQ_G], F32, tag="LGS")
            nc.scalar.activation(
                LGS[:].rearrange("p t g -> p (t g)"),
                GSL[:].rearrange("p t g -> p (t g)"),
                mybir.ActivationFunctionType.Ln)
            CEV = sm.tile([128, TC], F32, tag="CEV")
            nc.vector.tensor_reduce(CEV[:], LGS[:],
                                    mybir.AxisListType.X, mybir.AluOpType.add)
            nc.vector.tensor_sub(CEV[:], CEV[:], TSB[:])
            if flags["bsel"]:
                nc.vector.tensor_sub(CEV[:], CEV[:], BSL[:])
            nc.vector.tensor_scalar_mul(CEV[:], CEV[:], 0.25)

            # ce_s = ln(exp(sl0)+exp(sl1)) - (sl0 + st*(sl1-sl0))
            ES = sm.tile([128, TC, 2], F32, tag="ES")
            nc.scalar.activation(ES[:].rearrange("p t g -> p (t g)"),
                                 SLB[:].rearrange("p t g -> p (t g)"),
                                 mybir.ActivationFunctionType.Exp)
            CES = sm.tile([128, TC], F32, tag="CES")
            nc.vector.tensor_reduce(CES[:], ES[:],
                                    mybir.AxisListType.X, mybir.AluOpType.add)
            nc.scalar.activation(CES[:], CES[:], mybir.ActivationFunctionType.Ln)
            DD = sm.tile([128, TC], F32, tag="DD")
            nc.vector.tensor_sub(DD[:], SLB[:, :, 1], SLB[:, :, 0])
            nc.vector.tensor_tensor(DD[:], DD[:], STC[:], mybir.AluOpType.mult)
            nc.vector.tensor_add(DD[:], DD[:], SLB[:, :, 0])
            nc.vector.tensor_sub(CES[:], CES[:], DD[:])

            R4 = sm.tile([128, 4], F32, tag="R4")
            W1 = sm.tile([128, TC], F32, tag="W1")
            nc.vector.tensor_tensor(W1[:], CES[:], MSC[:], mybir.AluOpType.mult)
            W2 = sm.tile([128, TC], F32, tag="W2")
            nc.vector.tensor_tensor(W2[:], CEV[:], MVC[:], mybir.AluOpType.mult)
            for i, srcbuf in enumerate([W1, MSC, W2, MVC]):
                rtc = tr.tile([128, 1], F32, tag="rtc")
                nc.vector.tensor_reduce(rtc[:], srcbuf[:],
                                        mybir.AxisListType.X, mybir.AluOpType.add)
                nc.vector.tensor_copy(R4[:, i:i + 1], rtc[:])

            nc.sync.dma_start(d_out[:], R4[:])

    nc.compile()
    return nc


def prepare_inputs(inputs):
    """Host-side: fold LN into weights, apply d2b permutation, shard."""
    split = np.asarray(inputs["split"]).astype(np.int64)
    zq = np.asarray(inputs["zq"], dtype=np.float32)
    targets_vq = np.asarray(inputs["targets_vq"]).astype(np.int64)
    category = np.asarray(inputs["category"]).astype(np.int64)
    batch_id = np.asarray(inputs["batch_id"]).astype(np.int64)
    mask = np.asarray(inputs["mask"]).astype(bool)
    d2b = np.asarray(inputs["d2b"]).astype(np.int64)
    g = lambda k: np.asarray(inputs[k], dtype=np.float32)
    split_emb, class_emb = g("split_emb"), g("class_emb")
    vq_proj_w, vq_proj_b = g("vq_proj_w"), g("vq_proj_b")
    ln1_s, ln1_b = g("ln1_s"), g("ln1_b")
    qkv_w, qkv_b = g("qkv_w"), g("qkv_b")
    attn_w, attn_b = g("attn_w"), g("attn_b")
    ln2_s, ln2_b = g("ln2_s"), g("ln2_b")
    fc1_w, fc1_b = g("fc1_w"), g("fc1_b")
    fc2_w, fc2_b = g("fc2_w"), g("fc2_b")
    lnx_s, lnx_b = g("lnx_s"), g("lnx_b")
    split_w, split_b = g("split_w"), g("split_b")
    vq_w, vq_b = g("vq_w"), g("vq_b")

    # LN folds
    qkv_w_eff = ln1_s[:, :, None] * qkv_w                       # [L,C,3C]
    qkv_b_eff = np.einsum("lc,lcn->ln", ln1_b, qkv_w) + qkv_b   # [L,3C]
    fc1_w_eff = ln2_s[:, :, None] * fc1_w
    fc1_b_eff = np.einsum("lc,lcn->ln", ln2_b, fc1_w) + fc1_b
    vq_w_eff = lnx_s[:, None] * vq_w
    vq_b_eff = lnx_b @ vq_w + vq_b
    spl_w_eff = lnx_s[:, None] * split_w
    spl_b_eff = lnx_b @ split_w + split_b

    # token embedding pieces, depth order
    cond_rows = class_emb[category[batch_id]]                   # [N,C]
    base_depth = np.empty((N, C), np.float32)
    base_depth[:N_SPLIT] = split_emb[split]
    base_depth[N_SPLIT:] = vq_proj_b[None, :]
    base_depth[mask] = cond_rows[mask]
    zq_depth = np.zeros((N, DH), np.float32)
    zq_depth[N_SPLIT:] = zq
    zq_depth[mask] = 0.0

    ms_depth = np.zeros(N, np.float32)
    ms_depth[:N_SPLIT] = mask[:N_SPLIT]
    mv_depth = np.zeros(N, np.float32)
    mv_depth[N_SPLIT:] = mask[N_SPLIT:]
    st_depth = np.zeros(N, np.float32)
    st_depth[:N_SPLIT] = split
    wsel_depth = np.zeros((N, C), np.float32)
    cols = targets_vq + np.arange(VQ_G)[None, :] * VQ_SIZE      # [N_VQ,4]
    wsel_depth[N_SPLIT:] = vq_w_eff.T[cols].sum(axis=1)         # [N_VQ,C]
    bsel_depth = np.zeros(N, np.float32)
    bsel_depth[N_SPLIT:] = vq_b_eff[cols].sum(axis=1)

    # window order + positional embedding
    pe = _sin_pos_emb(N, C)
    emb_w = base_depth[d2b] + pe
    zq_w = zq_depth[d2b]
    ms_w, mv_w, st_w = ms_depth[d2b], mv_depth[d2b], st_depth[d2b]
    wsel_w, bsel_w = wsel_depth[d2b], bsel_depth[d2b]

    flags = {
        "bqkv": bool(np.any(qkv_b_eff[:, :2 * C])),
        "bqkv_v": bool(np.any(qkv_b_eff[:, 2 * C:])),
        "battn": bool(np.any(attn_b)),
        "bfc1": bool(np.any(fc1_b_eff)),
        "bfc2": bool(np.any(fc2_b)),
        "bspl": bool(np.any(spl_b_eff)),
        "bsel": bool(np.any(bsel_w)),
        "ebq": bool(np.any(vq_b_eff)),
    }

    shared = {
        "vqpw": vq_proj_w.astype(BF),
        "wqkv": qkv_w_eff.astype(BF),
        "wattn": attn_w.astype(BF),
        "wfc1": fc1_w_eff.astype(BF),
        "wfc2": fc2_w.astype(BF),
        "bqkv": qkv_b_eff.astype(np.float32),
        "battn": attn_b.astype(np.float32),
        "bfc1": fc1_b_eff.astype(np.float32),
        "bfc2": fc2_b.astype(np.float32),
        "wvq": vq_w_eff.astype(BF),
        "wspl": spl_w_eff.astype(BF),
        "bspl": spl_b_eff.astype(np.float32),
        "ebq": np.exp(vq_b_eff).astype(np.float32),
    }
    in_maps = []
    for c in range(NCORES):
        s = slice(c * T, (c + 1) * T)
        m = dict(shared)
        m["emb"] = np.ascontiguousarray(emb_w[s])
        m["zqt"] = np.ascontiguousarray(zq_w[s].T).astype(BF)
        m["wsel"] = wsel_w[s].astype(BF)
        m["bsel"] = np.ascontiguousarray(bsel_w[s])
        m["msc"] = np.ascontiguousarray(ms_w[s])
        m["mvc"] = np.ascontiguousarray(mv_w[s])
        m["stc"] = np.ascontiguousarray(st_w[s])
        in_maps.append(m)
    return in_maps, flags


def kernel(**inputs) -> np.ndarray:
    in_maps, flags = prepare_inputs(inputs)
    key = tuple(sorted(flags.items()))
    if key not in _CACHE:
        _CACHE[key] = build_nc(flags)
    nc = _CACHE[key]
    res = run_bass_kernel_spmd(nc, in_maps, core_ids=list(range(NCORES)))
    parts = np.stack([res.results[c]["out"].sum(axis=0) for c in range(NCORES)])
    s = parts.sum(axis=0)
    split_loss = s[0] / max(s[1], 1.0)
    vq_loss = s[2] / max(s[3], 1.0)
    return np.stack([split_loss, vq_loss]).astype(np.float32)


# revision 38
# speedup vs baseline: 1.0544x; 1.0544x over previous
"""Trainium2 Bass kernel for nn_MAR_52209622450490 (OctFormer sparse attention).

Sharding: the depth2batch gather is applied host-side while sharding — each
core gets a contiguous 2048-token slice of the *window-ordered* token stream.
2048 is a multiple of the 512-token super-window (P*DIL), so both the dense
and the dilated window partitions are fully core-local; no collectives needed.
The 4 transformer blocks plus both loss heads run on-device per core; each
core emits 4 partial sums (ce_s*ms, ms, ce_v*mv, mv) combined on host.

Device layout notes:
 - residual stream XB kept [token(part) x C(free)] f32; LN via bn_stats.
 - LN scale/bias folded into the following matmul's weights host-side.
 - matmul operands in bf16; channel-on-partition operands (HT/OT) produced
   by XBAR DMA transposes (contiguous dests) or PE transposes (dilated OT).
 - attention: scores-transposed S'[kq, p] via 4-head row-tiled K=32 matmuls
   (tile_position); exp on ACT (no max-subtraction needed: logits are O(0.1));
   AV matmul uses V with an appended ones-column so each output tile carries
   its softmax normalizer Z; normalization fused into PSUM evacuation.
 - dilated windows are pure strided APs on the token (free) axis.
 - PSUM banks are statically partitioned per pipeline stage (3 mm / 2 scores
   / 2 AV / 1 transpose) -- a single shared pool FIFO was the main
   in-flight-parallelism limiter (~600us -> ~480us in the cost model).
 - block-boundary critical prefix (LN1 chunks 0-3, QKV-proj nk0, V of the
   first window pair) is priority-backdated (tc.high_priority) so the Tile
   scheduler slots it into the previous block's MLP phase as soon as its
   per-chunk deps clear, shrinking the ACT idle bubble at block boundaries.
"""
import numpy as np
import ml_dtypes

import concourse.tile as tile
from concourse import bacc, mybir
from concourse.bass_utils import run_bass_kernel_spmd
from concourse.masks import make_identity

N_SPLIT = 4096
N_VQ = 12288
N = N_SPLIT + N_VQ
C = 256
H = 8
DH = 32
L = 4
P = 256
DIL = 2
HID = 4 * C
VQ_G = 4
VQ_SIZE = 256
NCORES = 8
T = N // NCORES            # 2048 tokens per core
TC = T // 128              # 16 row-tiles per core
NWIN = T // P              # 8 windows per core
EPS = 1e-5
SCALE = DH ** -0.5

F32 = mybir.dt.float32
BF16 = mybir.dt.bfloat16
BF = ml_dtypes.bfloat16

_CACHE = {}


def _sin_pos_emb(n, c):
    pos = np.arange(n, dtype=np.float32)[:, None]
    half = c // 2
    freqs = np.exp(-np.log(10000.0) * np.arange(half, dtype=np.float32) / half)
    ang = pos * freqs
    return np.concatenate([np.sin(ang), np.cos(ang)], axis=-1).astype(np.float32)



def _st(beg, cnt, step):
    return slice(beg, beg + (cnt - 1) * step + 1, step)

from contextlib import ExitStack as _ES

PRIO_OFF = 700

def build_nc(flags, n_blocks=L, dump=None):
    """flags: dict name->bool, whether each bias family is nonzero.
    dump: None|'xb'|'xn' adds a [T, C] f32 debug output."""
    nc = bacc.Bacc(None, target_bir_lowering=False)

    d_emb = nc.declare_dram_parameter("emb", [T, C], F32, isOutput=False)
    d_zqt = nc.declare_dram_parameter("zqt", [DH, T], BF16, isOutput=False)
    d_vqpw = nc.declare_dram_parameter("vqpw", [DH, C], BF16, isOutput=False)
    d_wqkv = nc.declare_dram_parameter("wqkv", [L, C, 3 * C], BF16, isOutput=False)
    d_wattn = nc.declare_dram_parameter("wattn", [L, C, C], BF16, isOutput=False)
    d_wfc1 = nc.declare_dram_parameter("wfc1", [L, C, HID], BF16, isOutput=False)
    d_wfc2 = nc.declare_dram_parameter("wfc2", [L, HID, C], BF16, isOutput=False)
    d_bqkv = nc.declare_dram_parameter("bqkv", [L, 3 * C], F32, isOutput=False)
    d_battn = nc.declare_dram_parameter("battn", [L, C], F32, isOutput=False)
    d_bfc1 = nc.declare_dram_parameter("bfc1", [L, HID], F32, isOutput=False)
    d_bfc2 = nc.declare_dram_parameter("bfc2", [L, C], F32, isOutput=False)
    d_wvq = nc.declare_dram_parameter("wvq", [C, VQ_G * VQ_SIZE], BF16, isOutput=False)
    d_wspl = nc.declare_dram_parameter("wspl", [C, 2], BF16, isOutput=False)
    d_bspl = nc.declare_dram_parameter("bspl", [2], F32, isOutput=False)
    d_ebq = nc.declare_dram_parameter("ebq", [VQ_G * VQ_SIZE], F32, isOutput=False)
    d_wsel = nc.declare_dram_parameter("wsel", [T, C], BF16, isOutput=False)
    d_bsel = nc.declare_dram_parameter("bsel", [T], F32, isOutput=False)
    d_msc = nc.declare_dram_parameter("msc", [T], F32, isOutput=False)
    d_mvc = nc.declare_dram_parameter("mvc", [T], F32, isOutput=False)
    d_stc = nc.declare_dram_parameter("stc", [T], F32, isOutput=False)
    d_out = nc.declare_dram_parameter("out", [128, 4], F32, isOutput=True)
    d_dbg = None
    if dump is not None:
        d_dbg = nc.declare_dram_parameter("dbg", [T, C], F32, isOutput=True)

    with tile.TileContext(nc) as tc:
        with (
            tc.tile_pool(name="big", bufs=1) as big,
            tc.tile_pool(name="wpool", bufs=2) as wp,
            tc.tile_pool(name="small", bufs=1) as sm,
            tc.tile_pool(name="trans", bufs=10) as tr,
            tc.tile_pool(name="ebpool", bufs=4) as ebp,
            tc.tile_pool(name="evpool", bufs=3) as evp,
            tc.tile_pool(name="psum", bufs=3, space="PSUM") as psp,
            tc.tile_pool(name="psum_sc", bufs=2, space="PSUM") as psc,
            tc.tile_pool(name="psum_av", bufs=2, space="PSUM") as pav,
            tc.tile_pool(name="psum_tr", bufs=1, space="PSUM") as ptr,
        ):
            XB = big.tile([128, TC, C], F32, tag="XB")
            HT = big.tile([128, 2, T], BF16, tag="HT")
            QT = big.tile([128, 2, T], BF16, tag="QT")
            KT = big.tile([128, 2, T], BF16, tag="KT")
            VB = big.tile([128, TC, H, DH + 1], BF16, tag="VB")
            OROW = big.tile([128, TC, C], BF16, tag="OROW")
            OT = big.tile([128, 2, T], BF16, tag="OT")
            GT = big.tile([128, HID // 128, T], BF16, tag="GT")
            XN = big.tile([128, TC, C], BF16, tag="XN")

            ident = sm.tile([128, 128], BF16, tag="ident")
            make_identity(nc, ident[:])
            epsT = sm.tile([128, 1], F32, tag="eps")
            nc.vector.memset(epsT[:], EPS)
            zqt = sm.tile([DH, T], BF16, tag="zqt")
            nc.sync.dma_start(zqt[:], d_zqt[:])
            vqpw = sm.tile([DH, C], BF16, tag="vqpw")
            nc.sync.dma_start(vqpw[:], d_vqpw[:])

            nc.vector.memset(VB[:, :, :, DH], 1.0)

            # ---------------- embed (per-chunk DMA so LN1 starts early) ----
            demb = d_emb.rearrange("(t p) c -> p t c", p=128)
            for t in range(TC):
                nc.sync.dma_start(XB[:, t, :], demb[:, t, :])
                ps = psp.tile([128, 512], F32, tag="bank")
                nc.tensor.matmul(ps[:, :C], zqt[:, t * 128:(t + 1) * 128],
                                 vqpw[:], start=True, stop=True)
                nc.vector.tensor_add(XB[:, t, :], XB[:, t, :], ps[:, :C])

            def layernorm_to(dst_bf, t):
                st6 = tr.tile([128, 6], F32, tag="bn6")
                nc.vector.bn_stats(st6[:], XB[:, t, :])
                mv2 = tr.tile([128, 2], F32, tag="bn2")
                nc.vector.bn_aggr(mv2[:], st6[:])
                rstd = tr.tile([128, 1], F32, tag="rstd")
                nc.scalar.activation(rstd[:], mv2[:, 1:2],
                                     mybir.ActivationFunctionType.Sqrt,
                                     bias=epsT[:])
                nc.vector.reciprocal(rstd[:], rstd[:])
                nc.vector.tensor_scalar(dst_bf, XB[:, t, :],
                                        mv2[:, 0:1], rstd[:],
                                        mybir.AluOpType.subtract,
                                        mybir.AluOpType.mult)

            def dma_transpose_pair(dstT, t128, src_128x256):
                """contiguous-dest transpose via XBAR DMA (both c-chunks)."""
                for cc in range(2):
                    nc.sync.dma_start_transpose(
                        dstT[:, cc, t128:t128 + 128],
                        src_128x256[:, cc * 128:(cc + 1) * 128])

            def transpose_pair(dstT, tok_ap_2x128, src_128x256):
                """dstT[:, cc, tok] = src[:, cc*128:+128].T for cc in 0,1; one evac."""
                pst = ptr.tile([128, 512], BF16, tag="ps_tr")
                nc.tensor.transpose(pst[:, 0:128], src_128x256[:, 0:128], ident[:])
                nc.tensor.transpose(pst[:, 128:256], src_128x256[:, 128:256], ident[:])
                nc.vector.tensor_copy(tok_ap_2x128,
                                      pst[:, :256].rearrange("p (k n) -> p k n", k=2))

            def prio(cond):
                s = _ES()
                if cond:
                    s.enter_context(tc.high_priority(offset=PRIO_OFF))
                return s

            # ---------------- transformer blocks ----------------
            # prologue: LN of the embedding for block 0 (h -> HT)
            for t in range(TC):
                hbf = tr.tile([128, C], BF16, tag="hbf")
                layernorm_to(hbf[:], t)
                dma_transpose_pair(HT, t * 128, hbf[:])
            for l in range(n_blocks):
                wqkv = wp.tile([128, 2, 3 * C], BF16, tag="wqkv")
                nc.sync.dma_start(wqkv[:], d_wqkv[l].rearrange("(k p) n -> p k n", p=128))
                wattn = wp.tile([128, 2, C], BF16, tag="wattn")
                nc.sync.dma_start(wattn[:], d_wattn[l].rearrange("(k p) n -> p k n", p=128))
                wfc1 = wp.tile([128, 2, HID], BF16, tag="wfc1")
                nc.sync.dma_start(wfc1[:], d_wfc1[l].rearrange("(k p) n -> p k n", p=128))
                wfc2 = wp.tile([128, HID // 128, C], BF16, tag="wfc2")
                nc.sync.dma_start(wfc2[:], d_wfc2[l].rearrange("(k p) n -> p k n", p=128))
                bqkv = vbb = abb = f1b = f2b = None
                if flags["bqkv"]:
                    bqkv = wp.tile([128, 4], F32, tag="bqkv")
                    nc.sync.dma_start(bqkv[:], d_bqkv[l, :2 * C].rearrange("(g p) -> p g", p=128))
                if flags["bqkv_v"]:
                    vbb = wp.tile([128, C], F32, tag="vbb")
                    nc.sync.dma_start(vbb[:], d_bqkv[l, 2 * C:].to_broadcast([128, C]))
                if flags["battn"]:
                    abb = wp.tile([128, C], F32, tag="abb")
                    nc.sync.dma_start(abb[:], d_battn[l].to_broadcast([128, C]))
                if flags["bfc1"]:
                    f1b = wp.tile([128, HID // 128], F32, tag="f1b")
                    nc.sync.dma_start(f1b[:], d_bfc1[l].rearrange("(g p) -> p g", p=128))
                if flags["bfc2"]:
                    f2b = wp.tile([128, C], F32, tag="f2b")
                    nc.sync.dma_start(f2b[:], d_bfc2[l].to_broadcast([128, C]))

                dil = DIL if (l % 2 == 1) else 1

                # QT / KT (transposed q,k; head-major on partitions)
                for nk in range(T // 512):
                    with prio(l > 0 and nk == 0):
                        for g in range(4):      # 0,1 -> Q ; 2,3 -> K
                            dstT = QT if g < 2 else KT
                            gg = g % 2
                            ps = psp.tile([128, 512], F32, tag="bank")
                            for cc in range(2):
                                nc.tensor.matmul(ps[:],
                                                 wqkv[:, cc, g * 128:(g + 1) * 128],
                                                 HT[:, cc, nk * 512:(nk + 1) * 512],
                                                 start=(cc == 0), stop=(cc == 1))
                            dsl = dstT[:, gg, nk * 512:(nk + 1) * 512]
                            if flags["bqkv"]:
                                nc.scalar.activation(dsl, ps[:],
                                                     mybir.ActivationFunctionType.Identity,
                                                     bias=bqkv[:, g:g + 1])
                            elif nk % 2 == 0:
                                nc.vector.tensor_copy(dsl, ps[:])
                            else:
                                nc.scalar.copy(dsl, ps[:])

                # V in window order: VB[:, wlin*2+qc, h, 0:DH]
                for wlin in range(NWIN):
                  with prio(l > 0 and wlin < 2):
                    sw, r = divmod(wlin, dil)
                    start = sw * P * dil + r
                    for qc in range(2):
                        ps = psp.tile([128, 512], F32, tag="bank")
                        tok = _st(start + qc * 128 * dil, 128, dil)
                        for cc in range(2):
                            nc.tensor.matmul(ps[:, :C], HT[:, cc, tok],
                                             wqkv[:, cc, 2 * C:3 * C],
                                             start=(cc == 0), stop=(cc == 1))
                        vdst = VB[:, wlin * 2 + qc, :, 0:DH]
                        psv = ps[:, :C].rearrange("p (h d) -> p h d", h=H)
                        if flags["bqkv_v"]:
                            nc.vector.tensor_tensor(
                                vdst, psv,
                                vbb[:].rearrange("p (h d) -> p h d", h=H),
                                mybir.AluOpType.add)
                        else:
                            nc.vector.tensor_copy(vdst, psv)

                # attention, window pairs: all scores (row-tiled PE mode)
                # for both windows first, then AV/transposes (full-array
                # mode) -- halves PE tiling-mode switch drains on silicon.
                def scores_for(wlin):
                    sw, r = divmod(wlin, dil)
                    start = sw * P * dil + r
                    alltok = _st(start, P, dil)
                    EB = ebp.tile([128, 2, H, P], BF16, tag="EB")
                    for g in range(2):
                        for hs in range(4):
                            hh = g * 4 + hs
                            prt = slice(hs * 32, (hs + 1) * 32)
                            pss = psc.tile([128, 512], F32, tag="ps_sc")
                            for qc in range(2):
                                ktok = _st(start + qc * 128 * dil, 128, dil)
                                nc.tensor.matmul(pss[:, qc * P:(qc + 1) * P],
                                                 KT[prt, g, ktok],
                                                 QT[prt, g, alltok],
                                                 start=True, stop=True,
                                                 tile_position=(hs * 32, 0))
                            nc.scalar.activation(
                                EB[:, :, hh, :],
                                pss[:].rearrange("p (q x) -> p q x", q=2),
                                mybir.ActivationFunctionType.Exp,
                                scale=SCALE)
                    return EB

                EBs = {}
                for wlin in range(NWIN):
                    if wlin % 2 == 0:
                        EBs[wlin] = scores_for(wlin)
                        EBs[wlin + 1] = scores_for(wlin + 1)
                    sw, r = divmod(wlin, dil)
                    start = sw * P * dil + r
                    EB = EBs[wlin]
                    for wc in range(2):
                        pso = pav.tile([128, H, DH + 1], F32, tag="ps_av")
                        for hh in range(H):
                            for qc in range(2):
                                nc.tensor.matmul(
                                    pso[:, hh, :],
                                    EB[:, qc, hh, wc * 128:(wc + 1) * 128],
                                    VB[:, wlin * 2 + qc, hh, :],
                                    start=(qc == 0), stop=(qc == 1))
                        rz = tr.tile([128, H], F32, tag="rz")
                        nc.vector.reciprocal(rz[:], pso[:, :, DH])
                        nc.vector.tensor_tensor(
                            OROW[:, wlin * 2 + wc, :].rearrange("p (h d) -> p h d", h=H),
                            pso[:, :, 0:DH],
                            rz[:, :, None].to_broadcast([128, H, DH]),
                            mybir.AluOpType.mult)
                        dtok = _st(start + wc * 128 * dil, 128, dil)
                        if dil == 1:
                            dma_transpose_pair(OT, start + wc * 128,
                                               OROW[:, wlin * 2 + wc, :])
                        else:
                            transpose_pair(OT, OT[:, :, dtok],
                                           OROW[:, wlin * 2 + wc, :])

                # proj + residual + LN2 + h2T, fused per t-chunk
                for t in range(TC):
                    ps = psp.tile([128, 512], F32, tag="bank")
                    for cc in range(2):
                        nc.tensor.matmul(ps[:, :C], OT[:, cc, t * 128:(t + 1) * 128],
                                         wattn[:, cc, :],
                                         start=(cc == 0), stop=(cc == 1))
                    nc.vector.tensor_add(XB[:, t, :], XB[:, t, :], ps[:, :C])
                    if flags["battn"]:
                        nc.vector.tensor_add(XB[:, t, :], XB[:, t, :], abb[:])
                    hbf = tr.tile([128, C], BF16, tag="hbf")
                    layernorm_to(hbf[:], t)
                    dma_transpose_pair(HT, t * 128, hbf[:])

                # fc1 + gelu -> GT (transposed, hid on partitions)
                for nk in range(T // 512):
                    for m in range(HID // 128):
                        ps = psp.tile([128, 512], F32, tag="bank")
                        for cc in range(2):
                            nc.tensor.matmul(ps[:],
                                             wfc1[:, cc, m * 128:(m + 1) * 128],
                                             HT[:, cc, nk * 512:(nk + 1) * 512],
                                             start=(cc == 0), stop=(cc == 1))
                        nc.scalar.activation(
                            GT[:, m, nk * 512:(nk + 1) * 512], ps[:],
                            mybir.ActivationFunctionType.Gelu_apprx_tanh,
                            bias=(f1b[:, m:m + 1] if flags["bfc1"] else 0.0))

                # fc2 + residual + next-block LN (or final LN into XN)
                last = (l == n_blocks - 1)
                for t in range(TC):
                    ps = psp.tile([128, 512], F32, tag="bank")
                    for hc in range(HID // 128):
                        nc.tensor.matmul(ps[:, :C], GT[:, hc, t * 128:(t + 1) * 128],
                                         wfc2[:, hc, :],
                                         start=(hc == 0), stop=(hc == HID // 128 - 1))
                    nc.vector.tensor_add(XB[:, t, :], XB[:, t, :], ps[:, :C])
                    if flags["bfc2"]:
                        nc.vector.tensor_add(XB[:, t, :], XB[:, t, :], f2b[:])
                    if last:
                        layernorm_to(XN[:, t, :], t)
                        dma_transpose_pair(HT, t * 128, XN[:, t, :])
                    else:
                        hbf = tr.tile([128, C], BF16, tag="hbf")
                        layernorm_to(hbf[:], t)
                        dma_transpose_pair(HT, t * 128, hbf[:])

            if dump == "xb":
                nc.sync.dma_start(d_dbg.rearrange("(t p) c -> p t c", p=128), XB[:])

            # ---------------- heads ----------------
            wvq = sm.tile([128, 2, VQ_G * VQ_SIZE], BF16, tag="wvq")
            nc.sync.dma_start(wvq[:], d_wvq.rearrange("(k p) n -> p k n", p=128))
            wspl = sm.tile([128, 2, 2], BF16, tag="wspl")
            nc.sync.dma_start(wspl[:], d_wspl.rearrange("(k p) n -> p k n", p=128))
            wselB = sm.tile([128, TC, C], BF16, tag="wsel")
            nc.sync.dma_start(wselB[:], d_wsel.rearrange("(t p) c -> p t c", p=128))
            MSC = sm.tile([128, TC], F32, tag="msc")
            nc.sync.dma_start(MSC[:], d_msc.rearrange("(t p) -> p t", p=128))
            MVC = sm.tile([128, TC], F32, tag="mvc")
            nc.sync.dma_start(MVC[:], d_mvc.rearrange("(t p) -> p t", p=128))
            STC = sm.tile([128, TC], F32, tag="stc")
            nc.sync.dma_start(STC[:], d_stc.rearrange("(t p) -> p t", p=128))
            if flags["bsel"]:
                BSL = sm.tile([128, TC], F32, tag="bsel")
                nc.sync.dma_start(BSL[:], d_bsel.rearrange("(t p) -> p t", p=128))
            if flags["ebq"]:
                EBQ = sm.tile([128, VQ_G * VQ_SIZE], F32, tag="ebq")
                nc.sync.dma_start(EBQ[:], d_ebq.to_broadcast([128, VQ_G * VQ_SIZE]))
            if flags["bspl"]:
                BSP = sm.tile([128, 2], F32, tag="bspl")
                nc.sync.dma_start(BSP[:], d_bspl.to_broadcast([128, 2]))

            SLB = sm.tile([128, TC, 2], F32, tag="SLB")
            GSL = sm.tile([128, TC, VQ_G], F32, tag="GSL")
            TSB = sm.tile([128, TC], F32, tag="TSB")

            # final LN -> XN (bf16) and XNT (reuse HT)
            if n_blocks == 0:
                for t in range(TC):
                    layernorm_to(XN[:, t, :], t)
                    dma_transpose_pair(HT, t * 128, XN[:, t, :])
            if dump == "xn":
                nc.gpsimd.dma_start(d_dbg.rearrange("(t p) c -> p t c", p=128), XN[:])

            for t in range(TC):
                EV = evp.tile([128, VQ_G * VQ_SIZE], F32, tag="EV")
                for nk in range(2):
                    ps = psp.tile([128, 512], F32, tag="bank")
                    for cc in range(2):
                        nc.tensor.matmul(ps[:],
                                         HT[:, cc, t * 128:(t + 1) * 128],
                                         wvq[:, cc, nk * 512:(nk + 1) * 512],
                                         start=(cc == 0), stop=(cc == 1))
                    nc.scalar.activation(EV[:, nk * 512:(nk + 1) * 512], ps[:],
                                         mybir.ActivationFunctionType.Exp)
                if flags["ebq"]:
                    nc.vector.tensor_tensor(EV[:], EV[:], EBQ[:],
                                            mybir.AluOpType.mult)
                g4 = tr.tile([128, VQ_G], F32, tag="g4")
                nc.vector.tensor_reduce(
                    g4[:],
                    EV[:].rearrange("p (g v) -> p g v", g=VQ_G),
                    mybir.AxisListType.X, mybir.AluOpType.add)
                nc.vector.tensor_copy(GSL[:, t, :], g4[:])

                ps2 = psp.tile([128, 512], F32, tag="bank")
                for cc in range(2):
                    nc.tensor.matmul(ps2[:, :2],
                                     HT[:, cc, t * 128:(t + 1) * 128],
                                     wspl[:, cc, :],
                                     start=(cc == 0), stop=(cc == 1))
                if flags["bspl"]:
                    nc.vector.tensor_tensor(SLB[:, t, :], ps2[:, :2], BSP[:],
                                            mybir.AluOpType.add)
                else:
                    nc.vector.tensor_copy(SLB[:, t, :], ps2[:, :2])

                tmp = tr.tile([128, C], F32, tag="wdot")
                nc.vector.tensor_tensor(tmp[:], XN[:, t, :], wselB[:, t, :],
                                        mybir.AluOpType.mult)
                rt1 = tr.tile([128, 1], F32, tag="rt1")
                nc.vector.tensor_reduce(rt1[:], tmp[:],
                                        mybir.AxisListType.X, mybir.AluOpType.add)
                nc.vector.tensor_copy(TSB[:, t:t + 1], rt1[:])

            # finish:  ce_v = 0.25*(sum_g ln GSL_g) - 0.25*(TSB [+bsel])
            LGS = sm.tile([128, TC, VQ_G], F32, tag="LGS")
            nc.scalar.activation(
                LGS[:].rearrange("p t g -> p (t g)"),
                GSL[:].rearrange("p t g -> p (t g)"),
                mybir.ActivationFunctionType.Ln)
            CEV = sm.tile([128, TC], F32, tag="CEV")
            nc.vector.tensor_reduce(CEV[:], LGS[:],
                                    mybir.AxisListType.X, mybir.AluOpType.add)
            nc.vector.tensor_sub(CEV[:], CEV[:], TSB[:])
            if flags["bsel"]:
                nc.vector.tensor_sub(CEV[:], CEV[:], BSL[:])
            nc.vector.tensor_scalar_mul(CEV[:], CEV[:], 0.25)

            # ce_s = ln(exp(sl0)+exp(sl1)) - (sl0 + st*(sl1-sl0))
            ES = sm.tile([128, TC, 2], F32, tag="ES")
            nc.scalar.activation(ES[:].rearrange("p t g -> p (t g)"),
                                 SLB[:].rearrange("p t g -> p (t g)"),
                                 mybir.ActivationFunctionType.Exp)
            CES = sm.tile([128, TC], F32, tag="CES")
            nc.vector.tensor_reduce(CES[:], ES[:],
                                    mybir.AxisListType.X, mybir.AluOpType.add)
            nc.scalar.activation(CES[:], CES[:], mybir.ActivationFunctionType.Ln)
            DD = sm.tile([128, TC], F32, tag="DD")
            nc.vector.tensor_sub(DD[:], SLB[:, :, 1], SLB[:, :, 0])
            nc.vector.tensor_tensor(DD[:], DD[:], STC[:], mybir.AluOpType.mult)
            nc.vector.tensor_add(DD[:], DD[:], SLB[:, :, 0])
            nc.vector.tensor_sub(CES[:], CES[:], DD[:])

            R4 = sm.tile([128, 4], F32, tag="R4")
            W1 = sm.tile([128, TC], F32, tag="W1")
            nc.vector.tensor_tensor(W1[:], CES[:], MSC[:], mybir.AluOpType.mult)
            W2 = sm.tile([128, TC], F32, tag="W2")
            nc.vector.tensor_tensor(W2[:], CEV[:], MVC[:], mybir.AluOpType.mult)
            for i, srcbuf in enumerate([W1, MSC, W2, MVC]):
                rtc = tr.tile([128, 1], F32, tag="rtc")
                nc.vector.tensor_reduce(rtc[:], srcbuf[:],
                                        mybir.AxisListType.X, mybir.AluOpType.add)
                nc.vector.tensor_copy(R4[:, i:i + 1], rtc[:])

            nc.sync.dma_start(d_out[:], R4[:])

    nc.compile()
    return nc


def prepare_inputs(inputs):
    """Host-side: fold LN into weights, apply d2b permutation, shard."""
    split = np.asarray(inputs["split"]).astype(np.int64)
    zq = np.asarray(inputs["zq"], dtype=np.float32)
    targets_vq = np.asarray(inputs["targets_vq"]).astype(np.int64)
    category = np.asarray(inputs["category"]).astype(np.int64)
    batch_id = np.asarray(inputs["batch_id"]).astype(np.int64)
    mask = np.asarray(inputs["mask"]).astype(bool)
    d2b = np.asarray(inputs["d2b"]).astype(np.int64)
    g = lambda k: np.asarray(inputs[k], dtype=np.float32)
    split_emb, class_emb = g("split_emb"), g("class_emb")
    vq_proj_w, vq_proj_b = g("vq_proj_w"), g("vq_proj_b")
    ln1_s, ln1_b = g("ln1_s"), g("ln1_b")
    qkv_w, qkv_b = g("qkv_w"), g("qkv_b")
    attn_w, attn_b = g("attn_w"), g("attn_b")
    ln2_s, ln2_b = g("ln2_s"), g("ln2_b")
    fc1_w, fc1_b = g("fc1_w"), g("fc1_b")
    fc2_w, fc2_b = g("fc2_w"), g("fc2_b")
    lnx_s, lnx_b = g("lnx_s"), g("lnx_b")
    split_w, split_b = g("split_w"), g("split_b")
    vq_w, vq_b = g("vq_w"), g("vq_b")

    # LN folds
    qkv_w_eff = ln1_s[:, :, None] * qkv_w                       # [L,C,3C]
    qkv_b_eff = np.einsum("lc,lcn->ln", ln1_b, qkv_w) + qkv_b   # [L,3C]
    fc1_w_eff = ln2_s[:, :, None] * fc1_w
    fc1_b_eff = np.einsum("lc,lcn->ln", ln2_b, fc1_w) + fc1_b
    vq_w_eff = lnx_s[:, None] * vq_w
    vq_b_eff = lnx_b @ vq_w + vq_b
    spl_w_eff = lnx_s[:, None] * split_w
    spl_b_eff = lnx_b @ split_w + split_b

    # token embedding pieces, depth order
    cond_rows = class_emb[category[batch_id]]                   # [N,C]
    base_depth = np.empty((N, C), np.float32)
    base_depth[:N_SPLIT] = split_emb[split]
    base_depth[N_SPLIT:] = vq_proj_b[None, :]
    base_depth[mask] = cond_rows[mask]
    zq_depth = np.zeros((N, DH), np.float32)
    zq_depth[N_SPLIT:] = zq
    zq_depth[mask] = 0.0

    ms_depth = np.zeros(N, np.float32)
    ms_depth[:N_SPLIT] = mask[:N_SPLIT]
    mv_depth = np.zeros(N, np.float32)
    mv_depth[N_SPLIT:] = mask[N_SPLIT:]
    st_depth = np.zeros(N, np.float32)
    st_depth[:N_SPLIT] = split
    wsel_depth = np.zeros((N, C), np.float32)
    cols = targets_vq + np.arange(VQ_G)[None, :] * VQ_SIZE      # [N_VQ,4]
    wsel_depth[N_SPLIT:] = vq_w_eff.T[cols].sum(axis=1)         # [N_VQ,C]
    bsel_depth = np.zeros(N, np.float32)
    bsel_depth[N_SPLIT:] = vq_b_eff[cols].sum(axis=1)

    # window order + positional embedding
    pe = _sin_pos_emb(N, C)
    emb_w = base_depth[d2b] + pe
    zq_w = zq_depth[d2b]
    ms_w, mv_w, st_w = ms_depth[d2b], mv_depth[d2b], st_depth[d2b]
    wsel_w, bsel_w = wsel_depth[d2b], bsel_depth[d2b]

    flags = {
        "bqkv": bool(np.any(qkv_b_eff[:, :2 * C])),
        "bqkv_v": bool(np.any(qkv_b_eff[:, 2 * C:])),
        "battn": bool(np.any(attn_b)),
        "bfc1": bool(np.any(fc1_b_eff)),
        "bfc2": bool(np.any(fc2_b)),
        "bspl": bool(np.any(spl_b_eff)),
        "bsel": bool(np.any(bsel_w)),
        "ebq": bool(np.any(vq_b_eff)),
    }

    shared = {
        "vqpw": vq_proj_w.astype(BF),
        "wqkv": qkv_w_eff.astype(BF),
        "wattn": attn_w.astype(BF),
        "wfc1": fc1_w_eff.astype(BF),
        "wfc2": fc2_w.astype(BF),
        "bqkv": qkv_b_eff.astype(np.float32),
        "battn": attn_b.astype(np.float32),
        "bfc1": fc1_b_eff.astype(np.float32),
        "bfc2": fc2_b.astype(np.float32),
        "wvq": vq_w_eff.astype(BF),
        "wspl": spl_w_eff.astype(BF),
        "bspl": spl_b_eff.astype(np.float32),
        "ebq": np.exp(vq_b_eff).astype(np.float32),
    }
    in_maps = []
    for c in range(NCORES):
        s = slice(c * T, (c + 1) * T)
        m = dict(shared)
        m["emb"] = np.ascontiguousarray(emb_w[s])
        m["zqt"] = np.ascontiguousarray(zq_w[s].T).astype(BF)
        m["wsel"] = wsel_w[s].astype(BF)
        m["bsel"] = np.ascontiguousarray(bsel_w[s])
        m["msc"] = np.ascontiguousarray(ms_w[s])
        m["mvc"] = np.ascontiguousarray(mv_w[s])
        m["stc"] = np.ascontiguousarray(st_w[s])
        in_maps.append(m)
    return in_maps, flags


def kernel(**inputs) -> np.ndarray:
    in_maps, flags = prepare_inputs(inputs)
    key = tuple(sorted(flags.items()))
    if key not in _CACHE:
        _CACHE[key] = build_nc(flags)
    nc = _CACHE[key]
    res = run_bass_kernel_spmd(nc, in_maps, core_ids=list(range(NCORES)))
    parts = np.stack([res.results[c]["out"].sum(axis=0) for c in range(NCORES)])
    s = parts.sum(axis=0)
    split_loss = s[0] / max(s[1], 1.0)
    vq_loss = s[2] / max(s[3], 1.0)
    return np.stack([split_loss, vq_loss]).astype(np.float32)
